# revision 7
# baseline (speedup 1.0000x reference)
"""Trainium2 Bass kernel for nn_NonLinearConv2d (softplus-pair conv + BatchNorm).

Math:
  patches = unfold(x, k=3, s=1, p=1)                      # (N, 27, 1024)
  t(u)    = softplus(u)^2,  u = (v - theta)/DENOM
  I[n,l,c] = ALPHA * sum_k [ t(u) - t(u - VD/DENOM) ]
  V = R_TIA * I  -> BatchNorm2d(train) over (N,H,W) per channel.

Device decomposition (per core, data-parallel over batch N=32 -> 4 images/core):
  softplus(u) = ln(1 + e^u) and e^u = e^((v-SHIFT)/D) * e^((SHIFT-theta)/D).
  - ACT computes E = exp((v-SHIFT)/D) once on the compact patch array.
  - PE broadcast-matmul (fp32r) forms e^u for 128 (k-pair, channel) slots at
    once: lhsT rows carry W = e^((SHIFT-theta)/D) in a delta pattern.
  - ACT computes lnA = Ln(e^u + 1), lnB = Ln(e^u * e^(-VD/D) + 1).
  - A custom fused DVE op computes diff = (lnA-lnB)*(lnA+lnB) = lnA^2 - lnB^2.
  - PE fold-matmul (fp32r, PSUM-accumulated over 14 k-pairs) contracts the 128
    slots to 64 channels with weight ALPHA*R_TIA.
  - BN stats (sum, sumsq) per channel reduce locally, AllReduce across the 8
    cores, then a single per-partition affine (ACT Identity with scale/bias
    vectors) applies BN.
"""

import numpy as np

import concourse.bass as bass
import concourse.bacc as bacc
import concourse.mybir as mybir
import concourse.tile as tile
import concourse.dve_ops as dve_ops
from concourse.bass_utils import run_bass_kernel_spmd
from concourse.dve_spec import C0, Spec, Src0, Src1, lower, _has_src1
from concourse.dve_uop import DveOpSpec
from concourse.dve_table_gen import dve_ver_for

F32 = mybir.dt.float32
F32R = mybir.dt.float32r
AF = mybir.ActivationFunctionType
ALU = mybir.AluOpType

ALPHA = 0.0005625
R_TIA = 0.1
DENOM = 2.0 * 1.5 * 0.025  # 0.075
V_D = 0.1
BN_EPS = 1e-5
SHIFT = 4.5
SCALE_B = float(np.exp(-V_D / DENOM))  # e^(-4/3)
A_OFF = 30.0                             # log-domain offset: Ln input <= e^(umax-30)
EB = float(np.exp(-A_OFF))

N_CORES = 8
N_PER = 4            # images per core
COUT = 64
K27 = 27
KP = 14              # k-pairs (27 -> 14 pairs, last half-empty)
L = 1024             # 32*32 positions per image
NL = N_PER * L       # 4096 positions per core
NTOT = 32 * L        # global positions per channel


# --------------------------------------------------------------------------- #
# Custom DVE op: out = (in0 - in1) * (in0 + in1) = in0^2 - in1^2
# --------------------------------------------------------------------------- #
_DIFF_SQ = None


def get_diff_sq_op():
    global _DIFF_SQ
    if _DIFF_SQ is not None:
        return _DIFF_SQ
    name = "DIFF_SQ_OFF_ANT"
    for o in dve_ops.OPS:
        if o.name == name:
            _DIFF_SQ = o
            return o
    spec = Spec(
        body=(Src0 - Src1) * (Src0 + Src1 + C0),
        reference=lambda in0, in1, s0, s1, imm2: (
            (in0.astype(np.float32) - in1) * (in0.astype(np.float32) + in1 + s0)
        ),
    )
    row = max(dve_ops._SUB_OPCODE_FOR_NAME.values()) + 1
    assert row < 0x20
    dve_ops._SUB_OPCODE_FOR_NAME[name] = row
    shas = {}
    for ver in ("v3", "v4"):
        compiled = DveOpSpec(
            name=name, opcode=row, uops=lower(spec, ver=ver), rd1_en=_has_src1(spec)
        )
        shas[ver] = compiled.sha(ver)
    op = dve_ops.DveOp(name, spec, subdim=False, uops_sha=shas)
    dve_ops.OPS.append(op)
    dve_ops.CUSTOM_DVE_SPECS[name] = spec
    _DIFF_SQ = op
    return op


# --------------------------------------------------------------------------- #
# Device program
# --------------------------------------------------------------------------- #
def _register_const(nc, value):
    key = (F32, float(value))
    if key in nc.const_aps.aps:
        return
    t = nc.alloc_sbuf_tensor(f"const-float32-{value}", [128, 1], F32)
    nc.gpsimd.memset(t.ap(), float(value))
    nc.const_aps.aps[key] = t.ap()


def build_program(no_collective=False):
    diff_sq = get_diff_sq_op()
    nc = bacc.Bacc("TRN2", target_bir_lowering=False, debug=False,
                   num_devices=N_CORES)
    _register_const(nc, -SHIFT / DENOM)
    _register_const(nc, BN_EPS)
    _register_const(nc, EB)
    nc.all_engine_barrier()

    x_d = nc.dram_tensor("x", [N_PER, 3, 32, 32], F32, kind="ExternalInput")
    wb_d = nc.dram_tensor("wb", [2 * KP, KP * 128], F32R, kind="ExternalInput")
    fl_d = nc.dram_tensor("fl", [128, COUT], F32R, kind="ExternalInput")
    gb_d = nc.dram_tensor("gb", [COUT, 2], F32, kind="ExternalInput")
    y_d = nc.dram_tensor("y", [N_PER, COUT, 32, 32], F32, kind="ExternalOutput")

    with tile.TileContext(nc) as tc:
        with tc.tile_pool(name="const", bufs=1) as cpool, \
             tc.tile_pool(name="work", bufs=1) as wpool, \
             tc.tile_pool(name="ln", bufs=3) as lnpool, \
             tc.tile_pool(name="df", bufs=3) as dfpool, \
             tc.tile_pool(name="eps", bufs=2, space="PSUM") as epool, \
             tc.tile_pool(name="vps", bufs=2, space="PSUM") as vpool, \
             tc.tile_pool(name="dram", bufs=1, space="DRAM") as dpool:

            # ---- constants ------------------------------------------------ #
            wb_sb = cpool.tile([2 * KP, KP * 128], F32R, tag="wb")
            nc.sync.dma_start(wb_sb[:], wb_d.ap())
            fl_sb = cpool.tile([128, COUT], F32R, tag="fl")
            nc.sync.dma_start(fl_sb[:], fl_d.ap())
            gb_sb = cpool.tile([COUT, 2], F32, tag="gb")
            nc.sync.dma_start(gb_sb[:], gb_d.ap())

            # ---- unfold: 27 shifted-window DMAs into v_pack --------------- #
            # v_pack[4*k + img, y*32 + x] = xpad[img, ci(k), y+di-1, x+dj-1]
            v_pack = wpool.tile([4 * (K27 + 1), L], F32, tag="vpack")
            nc.vector.memset(v_pack[:], 0.0)
            x_ap = x_d.ap()
            for k in range(K27):
                ci, r = divmod(k, 9)
                di, dj = divmod(r, 3)
                y0, y1 = max(0, 1 - di), min(32, 33 - di)
                xx0, xx1 = max(0, 1 - dj), min(32, 33 - dj)
                src = x_ap[:, ci, y0 + di - 1:y1 + di - 1, xx0 + dj - 1:xx1 + dj - 1]
                dst3 = v_pack.rearrange("p (y x) -> p y x", x=32)
                dst = dst3[4 * k:4 * k + 4, y0:y1, xx0:xx1]
                nc.sync.dma_start(dst, src)

            # ---- E = exp((v - SHIFT)/DENOM), written as fp32r ------------- #
            e_pack = wpool.tile([4 * (K27 + 1), L], F32R, tag="epack")
            nc.scalar.activation(e_pack[:], v_pack[:], AF.Exp,
                                 bias=-SHIFT / DENOM, scale=1.0 / DENOM)

            # ---- re-layout to E_sb[k, img*L + l] -------------------------- #
            e_sb = cpool.tile([2 * KP, NL], F32R, tag="esb")
            for k in range(2 * KP):
                nc.sync.dma_start(
                    e_sb[k:k + 1, :],
                    e_pack[4 * k:4 * k + 4, :],
                )

            # ---- main loop ------------------------------------------------ #
            v_sb = cpool.tile([COUT, NL], F32, tag="vsb")
            for img in range(N_PER):
                v_ps = vpool.tile([COUT, L], F32, tag="vps")
                for kp in range(KP):
                    e_ps = epool.tile([128, L], F32, tag="eps")
                    lhs = wb_sb[:, 128 * kp:128 * (kp + 1)]
                    rhs = e_sb[:, img * L:(img + 1) * L]
                    for h in range(2):
                        nc.tensor.matmul(
                            e_ps[:, 512 * h:512 * (h + 1)], lhs,
                            rhs[:, 512 * h:512 * (h + 1)],
                            start=True, stop=True)
                    ln2 = lnpool.tile([128, 2 * L], F32, tag="ln2")
                    nc.scalar.activation(ln2[:, 0:L], e_ps[:], AF.Ln,
                                         bias=EB, scale=1.0)
                    nc.scalar.activation(ln2[:, L:2 * L], e_ps[:], AF.Ln,
                                         bias=EB, scale=SCALE_B)
                    diff = dfpool.tile([128, L], F32R, tag="diff")
                    nc.vector._custom_dve(diff_sq, out=diff[:],
                                          in0=ln2[:, 0:L], in1=ln2[:, L:2 * L],
                                          s0=2.0 * A_OFF)
                    for h in range(2):
                        nc.tensor.matmul(
                            v_ps[:, 512 * h:512 * (h + 1)], fl_sb[:],
                            diff[:, 512 * h:512 * (h + 1)],
                            start=(kp == 0), stop=(kp == KP - 1))
                nc.scalar.copy(v_sb[:, img * L:(img + 1) * L], v_ps[:])

            # ---- local BN stats ------------------------------------------- #
            s_t = wpool.tile([COUT, 1], F32, tag="s")
            nc.vector.reduce_sum(s_t[:], v_sb[:], axis=mybir.AxisListType.X)
            junk = wpool.tile([COUT, NL], F32, tag="junk")
            ss_t = wpool.tile([COUT, 1], F32, tag="ss")
            nc.scalar.activation(junk[:], v_sb[:], AF.Square, accum_out=ss_t[:])

            st4 = wpool.tile([COUT, 4], F32, tag="st4")
            nc.vector.memset(st4[:], 0.0)
            nc.vector.tensor_copy(st4[:, 0:1], s_t[:])
            nc.vector.tensor_copy(st4[:, 1:2], ss_t[:])

            # ---- AllReduce over the 8 cores ------------------------------- #
            cc_in = dpool.tile([COUT, 4], F32)
            cc_out = dpool.tile([COUT, 4], F32)
            nc.sync.dma_start(cc_in[:], st4[:])
            if no_collective:
                nc.sync.dma_start(cc_out[:], cc_in[:])
            else:
                nc.gpsimd.collective_compute(
                    "AllReduce", ALU.add,
                    replica_groups=[list(range(N_CORES))],
                    ins=[cc_in.opt()], outs=[cc_out.opt()])
            gst = wpool.tile([COUT, 4], F32, tag="gst")
            nc.sync.dma_start(gst[:], cc_out[:])

            # ---- BN scalars: a = gamma*rstd, b = beta - mean*a ------------ #
            mean_t = wpool.tile([COUT, 1], F32, tag="mean")
            nc.vector.tensor_scalar_mul(mean_t[:], gst[:, 0:1], 1.0 / NTOT)
            ms_t = wpool.tile([COUT, 1], F32, tag="ms")
            nc.vector.tensor_scalar_mul(ms_t[:], gst[:, 1:2], 1.0 / NTOT)
            nvar_t = wpool.tile([COUT, 1], F32, tag="nvar")
            nc.vector.scalar_tensor_tensor(nvar_t[:], mean_t[:], mean_t[:],
                                           ms_t[:], op0=ALU.mult,
                                           op1=ALU.subtract)
            lnv_t = wpool.tile([COUT, 1], F32, tag="lnv")
            nc.scalar.activation(lnv_t[:], nvar_t[:], AF.Ln,
                                 bias=BN_EPS, scale=-1.0)
            rstd_t = wpool.tile([COUT, 1], F32, tag="rstd")
            nc.scalar.activation(rstd_t[:], lnv_t[:], AF.Exp,
                                 bias=0.0, scale=-0.5)
            a_t = wpool.tile([COUT, 1], F32, tag="a")
            nc.vector.tensor_mul(a_t[:], gb_sb[:, 0:1], rstd_t[:])
            ma_t = wpool.tile([COUT, 1], F32, tag="ma")
            nc.vector.tensor_mul(ma_t[:], mean_t[:], a_t[:])
            b_t = wpool.tile([COUT, 1], F32, tag="b")
            nc.vector.scalar_tensor_tensor(b_t[:], gb_sb[:, 1:2], 1.0,
                                           ma_t[:], op0=ALU.mult,
                                           op1=ALU.subtract)

            # ---- apply + store -------------------------------------------- #
            vh = wpool.tile([COUT, NL], F32, tag="vh")
            nc.scalar.activation(vh[:], v_sb[:], AF.Identity,
                                 bias=b_t[:], scale=a_t[:])
            y_ap = y_d.ap()
            for img in range(N_PER):
                nc.sync.dma_start(y_ap[img], vh[:, img * L:(img + 1) * L])

    nc.compile()
    return nc


def host_inputs(x, theta, gamma, beta):
    """Per-core input maps (host-side weight folding)."""
    theta = np.asarray(theta, np.float64)
    w = np.exp((SHIFT - theta) / DENOM - A_OFF).astype(np.float32)  # (27, 64)
    wb = np.zeros((2 * KP, KP * 128), np.float32)
    for k in range(K27):
        kp, km = divmod(k, 2)
        wb[k, 128 * kp + 64 * km:128 * kp + 64 * km + 64] = w[k]
    fl = np.zeros((128, COUT), np.float32)
    ar = np.float32(ALPHA * R_TIA)
    fl[np.arange(64), np.arange(64)] = ar
    fl[64 + np.arange(64), np.arange(64)] = ar
    gb = np.stack([np.asarray(gamma, np.float32),
                   np.asarray(beta, np.float32)], axis=1)
    x = np.asarray(x, np.float32)
    return [
        {"x": np.ascontiguousarray(x[N_PER * c:N_PER * (c + 1)]),
         "wb": wb, "fl": fl, "gb": gb}
        for c in range(N_CORES)
    ]


_PROG = None


def get_program():
    global _PROG
    if _PROG is None:
        _PROG = build_program()
    return _PROG


def kernel(x, theta, gamma, beta):
    nc = get_program()
    ins = host_inputs(x, theta, gamma, beta)
    res = run_bass_kernel_spmd(nc, ins, core_ids=list(range(N_CORES)))
    return np.concatenate([res.results[c]["y"] for c in range(N_CORES)], axis=0)
